# revision 31
# baseline (speedup 1.0000x reference)
"""Trainium2 Bass kernel for nn_CrossAttention_15418932593009.

Reference computation (fp32):
    q = (x @ wq1) @ wq2                      # (b, n, h*d), bottleneck 40
    k = silu(x @ wk1) @ wk2
    v = (x @ wv1) @ wv2
    split '(b n (h d)) -> (b (h n) d)'       # heads folded into sequence!
    sim  = q @ k.T * d**-0.5                 # (b, h*n, h*n) = (4, 8192, 8192)
    attn = softmax(sim, axis=-1)
    out  = attn @ v                          # (b, h*n, d)
    merge back -> (b, n, h*d); out @ wo + bo

Sharding: 8 cores = 4 batches x 2 query-head groups (heads 0-3 / 4-7).
Each core computes full K/V for its batch (all 8 heads) and attention for
its 4 query heads (4096 query rows x 8192 keys), then its partial of the
output projection. Host sums the two partials per batch and adds bo.

Per-core design (cost-model-driven):
- Scores S^T tiles [128 keys, 512 queries] from fp32r matmuls (the q
  projection is pre-scaled by d**-0.5 * log2(e) on the host, so scores
  arrive in base-2).
- exp is split across two engines: ~70% of score tiles on ACT
  (activation Exp, scale=ln2, bf16 out) and ~30% on DVE via a two-pass
  Schraudolph: pass1 = tensor_scalar int32 writeback y=int(s*2^23+bias)
  whose bits reinterpret as 2^k*(1+f); pass2 = one custom DVE op that
  extracts m=1+f with AND/OR bit ops and multiplies by a deg-2
  correction poly 2^(m-1)/m (max rel err ~3.5e-3, validated on HW).
- A@V runs transposed-free-dim: out[128 queries, 65] = P^T.T @ V with
  bf16 P^T slices as the stationary operand and V (with a ones column
  for the softmax denominator) as the 65-wide moving operand; 64 key
  chunks accumulate into one PSUM bank holding all 4 query-chunk
  accumulators (single start/stop per bank).
- Normalize on DVE (per-partition reciprocal scalar), transpose o via
  PE transpose-mode, final projection as natural-layout [pos, 256]
  matmuls; output DMA'd untransposed.
"""

import numpy as np

HEADS = 8
D = 64
BOT = 40
B = 4
N = 1024
QS = 256
INNER = HEADS * D          # 512
GH = 4                     # query heads per core
KCH = HEADS * N // 128     # 64 key chunks of 128
NQT = 8                    # 8 query tiles of 512 per core
NCORES = 8

LOG2E = float(np.log2(np.e))
LN2 = float(np.log(2.0))
MAGIC = float(127 * (1 << 23))     # Schraudolph bias (bits of 1.0f)
SCALE23 = float(1 << 23)
# deg-2 minimax of corr(m) = 2^(m-1)/m on [1,2): out = ((A2*m+A1)*m+A0)*P~
CA2, CA1, CA0 = 0.23375693, -0.69456113, 1.45744953

# exp tile engine mix: DVE1 = 1-pass bf16 Schraudolph (cheap, ~1.8% rms),
# DVE2 = 2-pass corrected Schraudolph (~0.35% max), rest on ACT (exact).
MAGIC16 = float(127 * 128 - 7.0)   # bias calibrated for zero-mean ripple


def _exp_mode(i):
    if i % 4 == 1 or i % 16 == 2:
        return 1   # DVE 1-pass
    if i % 16 == 10:
        return 2   # DVE 2-pass
    return 0       # ACT

_BUILT = {}


def _register_exp_op():
    """Register the Schraudolph-correction custom DVE op (idempotent)."""
    import concourse.dve_ops as dve_ops
    from concourse.dve_spec import (
        AluOp, Bin, C0, C1, C2, C3, One, Spec, Src0,
        _has_src1, _spill_c3_to_src1, lower,
    )
    from concourse.dve_table_gen import dve_ver_for
    from concourse.dve_uop import DveOpSpec

    NAME = "EXP_SCHRAUD_CORR_ANT"
    if NAME in dve_ops._SUB_OPCODE_FOR_NAME:
        return next(op for op in dve_ops.OPS if op.name == NAME)

    # m = bitcast((bits(P~) & 0x007FFFFF) | bits(1.0)) = 1 + frac in [1,2)
    m = Bin(AluOp.BITWISE_OR, Bin(AluOp.BITWISE_AND, Src0, C0), One)
    body = _spill_c3_to_src1(((C1 * m + C2) * m + C3) * Src0)

    def ref(in0, in1, c0, c1, c2):
        bits = in0.view(np.int32)
        mm = ((bits & np.int32(0x007FFFFF)) | np.int32(0x3F800000)).view(
            np.float32
        )
        return ((c1 * mm + c2) * mm + in1) * in0

    spec = Spec(body=body, reference=ref)
    row = max(dve_ops._SUB_OPCODE_FOR_NAME.values()) + 1
    assert row < 0x20
    ver = dve_ver_for("TRN2")
    lowered = DveOpSpec(name=NAME, opcode=row, uops=lower(spec, ver=ver),
                        rd1_en=_has_src1(spec))
    op = dve_ops.DveOp(NAME, spec, subdim=False, uops_sha={ver: lowered.sha(ver)})
    dve_ops.OPS.append(op)
    dve_ops._SUB_OPCODE_FOR_NAME[NAME] = row
    dve_ops.CUSTOM_DVE_SPECS[NAME] = spec
    return op


def _build():
    """Build the single-core Bass module (same NEFF for all 8 cores)."""
    import concourse.bass as bass
    import concourse.mybir as mybir
    import concourse.tile as tile
    from concourse import bacc

    exp_op = _register_exp_op()

    dt = mybir.dt
    f32 = dt.float32
    f32r = dt.float32r
    bf16 = dt.bfloat16
    i32 = dt.int32
    AF = mybir.ActivationFunctionType
    Alu = mybir.AluOpType
    PSUM = bass.MemorySpace.PSUM

    nc = bacc.Bacc()

    # ---- DRAM I/O (per core); float32r = same bits as fp32 ----
    xT = nc.dram_tensor("xT", [QS, N], f32r, kind="ExternalInput")     # x[b].T
    wq1 = nc.dram_tensor("wq1", [QS, BOT], f32r, kind="ExternalInput")  # prescaled
    wk1 = nc.dram_tensor("wk1", [QS, BOT], f32r, kind="ExternalInput")
    wv1 = nc.dram_tensor("wv1", [QS, BOT], f32r, kind="ExternalInput")
    wq2g = nc.dram_tensor("wq2g", [BOT, GH * D], f32r, kind="ExternalInput")
    wk2 = nc.dram_tensor("wk2", [BOT, INNER], f32r, kind="ExternalInput")
    wv2 = nc.dram_tensor("wv2", [BOT, INNER], f32r, kind="ExternalInput")
    # wog: rows 0:64 and 64:128 both hold wo[g] head hl slice (duplicated so
    # odd-parity oT tiles at base partition 64 have matching-base weights)
    wog = nc.dram_tensor("wog", [128, GH, QS], dt.bfloat16, kind="ExternalInput")
    out = nc.dram_tensor("out", [N, QS], f32, kind="ExternalOutput")  # natural

    with tile.TileContext(nc) as tc:
        with (
            tc.tile_pool(name="consts", bufs=1) as consts,
            tc.tile_pool(name="bigs", bufs=1) as bigs,
            tc.tile_pool(name="pp", bufs=8) as ppool,
            tc.tile_pool(name="yy", bufs=6) as ypool,
            tc.tile_pool(name="yy2", bufs=2) as ypool2,
            tc.tile_pool(name="small", bufs=8) as small,
            tc.tile_pool(name="sp", bufs=3, space=PSUM) as sppool,
            tc.tile_pool(name="avp", bufs=2, space=PSUM) as avpool,
        ):
            # ---- load inputs: x on SP's DMA queue, weights on ACT's and
            # DVE's queues so the three transfers run concurrently
            xT_sb = bigs.tile([128, 2, N], f32r)
            for cc in range(2):
                nc.sync.dma_start(
                    xT_sb[:, cc, :],
                    xT.rearrange("(c p) n -> p c n", c=2)[:, cc, :],
                )
            w1_sb = {}
            for name, t in (("q", wq1), ("k", wk1), ("v", wv1)):
                w = consts.tile([128, 2, BOT], f32r, name=f"w{name}1_sb")
                nc.scalar.dma_start(w[:], t.rearrange("(c p) n -> p c n", c=2))
                w1_sb[name] = w
            wq2_sb = consts.tile([BOT, GH * D], f32r)
            nc.scalar.dma_start(wq2_sb[:], wq2g[:])
            wk2_sb = consts.tile([BOT, INNER], f32r)
            nc.vector.dma_start(wk2_sb[:], wk2[:])
            wv2_sb = consts.tile([BOT, INNER], f32r)
            nc.vector.dma_start(wv2_sb[:], wv2[:])
            wog_sb = consts.tile([128, GH, QS], dt.bfloat16)
            nc.vector.dma_start(wog_sb[:], wog[:])

            mask_sb = consts.tile([128, 1], i32)
            nc.vector.memset(mask_sb[:], 0x007FFFFF)
            a0_sb = consts.tile([128, 1], f32)
            nc.vector.memset(a0_sb[:], CA0)

            # copies out of PSUM alternate between ACT and DVE so neither
            # engine's serial chain gates PSUM buffer reuse
            copy_flip = [0]

            def psum_copy(dst, src):
                copy_flip[0] ^= 1
                if copy_flip[0]:
                    nc.scalar.activation(dst, src, AF.Copy)
                else:
                    nc.vector.tensor_copy(dst, src)

            # ---- bottleneck projections: bX^T = wX1^T @ x^T  (40, 1024) ----
            b_sb = {}
            for name in ("q", "k", "v"):
                ps = sppool.tile([128, 1024], f32, tag="sp")
                for s in range(2):
                    for cc in range(2):
                        nc.tensor.matmul(
                            ps[0:BOT, 512 * s : 512 * (s + 1)],
                            w1_sb[name][:, cc, :],
                            xT_sb[:, cc, 512 * s : 512 * (s + 1)],
                            start=(cc == 0),
                            stop=(cc == 1),
                        )
                bt = bigs.tile([BOT, N], f32r, name=f"b{name}_sb")
                if name == "k":
                    # silu(x) = x * sigmoid(x)
                    sg = bigs.tile([BOT, N], f32, name="sg_sb")
                    nc.scalar.activation(sg[:], ps[0:BOT, 0:N], AF.Sigmoid)
                    nc.vector.tensor_mul(bt[:], ps[0:BOT, 0:N], sg[:])
                else:
                    psum_copy(bt[:], ps[0:BOT, 0:N])
                b_sb[name] = bt

            qT_sb = bigs.tile([D, GH * N], f32r)
            kT_sb = bigs.tile([D, HEADS * N], f32r)

            def q_proj(hl):
                ps = sppool.tile([128, 1024], f32, tag="sp")
                for s in range(2):
                    nc.tensor.matmul(
                        ps[0:D, 512 * s : 512 * (s + 1)],
                        wq2_sb[:, D * hl : D * (hl + 1)],
                        b_sb["q"][:, 512 * s : 512 * (s + 1)],
                    )
                psum_copy(qT_sb[:, N * hl : N * (hl + 1)], ps[0:D, 0:N])

            def k_proj(hk):
                ps = sppool.tile([128, 1024], f32, tag="sp")
                for s in range(2):
                    nc.tensor.matmul(
                        ps[0:D, 512 * s : 512 * (s + 1)],
                        wk2_sb[:, D * hk : D * (hk + 1)],
                        b_sb["k"][:, 512 * s : 512 * (s + 1)],
                    )
                psum_copy(kT_sb[:, N * hk : N * (hk + 1)], ps[0:D, 0:N])

            # v natural (128 keys, kch, d+1) bf16 with ones column
            v_sb = bigs.tile([128, KCH, D + 1], bf16)
            nc.vector.memset(v_sb[:, :, D : D + 1], 1.0)
            vv = v_sb.rearrange("p (h pb) e -> p pb h e", pb=8)

            def v_proj(pb):
                ps = sppool.tile([128, 1024], f32, tag="sp")
                nc.tensor.matmul(
                    ps[:, 0:INNER],
                    b_sb["v"][:, 128 * pb : 128 * (pb + 1)],
                    wv2_sb[:],
                )
                psum_copy(
                    vv[:, pb, :, 0:D],
                    ps[:, 0:INNER].rearrange("p (h e) -> p h e", h=HEADS),
                )

            # heads in S-stream consumption order; v interleaved
            q_proj(0)
            for hk in range(HEADS):
                k_proj(hk)
                if hk < 4:
                    v_proj(2 * hk)
                    v_proj(2 * hk + 1)
            for hl in range(1, GH):
                q_proj(hl)

            # ---- attention stream ----
            o_sb = bigs.tile([128, 4 * NQT, D], bf16)   # normalized o, natural
            # oT: rows 0:64 = d of even qcg, 64:128 = d of odd qcg;
            # cols = 128*(qcg//2) + pos
            oT_sb = bigs.tile([128, 16 * 128], bf16)
            out_sb = bigs.tile([128, 8, QS], f32)

            def ep_recip(qt, av):
                rq = small.tile([128, 4, 1], f32, tag="rq")
                av4 = av.rearrange("p (q e) -> p q e", q=4)
                nc.vector.reciprocal(rq[:], av4[:, :, D : D + 1])
                return rq

            def ep_mul(qt, av, rq, qc):
                hl, s = divmod(qt, 2)
                qcg = hl * 8 + s * 4 + qc
                nc.vector.tensor_scalar(
                    o_sb[:, qcg, :],
                    av[:, 65 * qc : 65 * qc + D],
                    rq[:, qc, :],
                    None,
                    op0=Alu.mult,
                )

            def ep_transpose(qt, pair):
                # DMA-transpose two query chunks at once: o [128 pos, 128] ->
                # oT [128, 128] whose rows are (qc&1, d)
                hl, s = divmod(qt, 2)
                qcg = hl * 8 + s * 4 + 2 * pair
                nc.sync.dma_start_transpose(
                    oT_sb[:, 128 * (qcg // 2) : 128 * (qcg // 2) + 128],
                    o_sb[:, qcg : qcg + 2, :],
                )

            def final_block(pb):
                fp = sppool.tile([128, 1024], f32, tag="sp")
                half = (pb % 2) * D        # oT/wog row base by qcg parity
                for hl in range(GH):
                    qcg = hl * 8 + pb
                    nc.tensor.matmul(
                        fp[:, 0:QS],
                        oT_sb[half : half + D,
                              128 * (qcg // 2) : 128 * (qcg // 2) + 128],
                        wog_sb[half : half + D, hl, :],
                        start=(hl == 0),
                        stop=(hl == GH - 1),
                    )
                nc.vector.tensor_copy(out_sb[:, pb, :], fp[:, 0:QS])
                nc.sync.dma_start(
                    out[128 * pb : 128 * (pb + 1), :], out_sb[:, pb, :]
                )

            from collections import deque

            AV_LAG = 4  # tiles of pipeline lag before A@V consumes P

            def emit_av(work):
                av, pt, p = work
                for j in range(2):
                    c = 2 * p + j
                    for qc in range(4):
                        nc.tensor.matmul(
                            av[:, 65 * qc : 65 * qc + D + 1],
                            pt[:, 512 * j + 128 * qc :
                               512 * j + 128 * qc + 128],
                            v_sb[:, c, :],
                            start=(c == 0 and qc == 0),
                            stop=(c == KCH - 1 and qc == 3),
                            skip_group_check=True,
                        )

            av_work = deque()
            pending = None   # (qt, av) awaiting epilogue
            ep_rq = None
            for qt in range(NQT):
                hl, s = divmod(qt, 2)
                qcol = N * hl + 512 * s
                avt = avpool.tile([128, 512], f32, tag="av")
                av = avt[:, 0 : 4 * (D + 1)]
                for p in range(32):
                    i = qt * 32 + p
                    sp = sppool.tile([128, 1024], f32, tag="sp")
                    for j in range(2):
                        c = 2 * p + j
                        nc.tensor.matmul(
                            sp[:, 512 * j : 512 * (j + 1)],
                            kT_sb[:, 128 * c : 128 * (c + 1)],
                            qT_sb[:, qcol : qcol + 512],
                        )
                    mode = _exp_mode(i)
                    if mode == 1:
                        y16 = ypool.tile([128, 1024], dt.int16, tag="y16")
                        nc.vector.tensor_scalar(
                            y16[:], sp[:], 128.0, MAGIC16,
                            op0=Alu.mult, op1=Alu.add,
                        )
                        pt = y16.bitcast(bf16)
                    elif mode == 2:
                        y = ypool2.tile([128, 1024], i32, tag="y")
                        pt_t = ppool.tile([128, 1024], bf16, tag="P")
                        nc.vector.tensor_scalar(
                            y[:], sp[:], SCALE23, MAGIC,
                            op0=Alu.mult, op1=Alu.add,
                        )
                        nc.vector._custom_dve(
                            exp_op, out=pt_t[:], in0=y.bitcast(f32)[:],
                            in1=a0_sb[:], s0=mask_sb.bitcast(f32)[:],
                            s1=CA2, imm2=CA1,
                        )
                        pt = pt_t
                    else:
                        pt_t = ppool.tile([128, 1024], bf16, tag="P")
                        nc.scalar.activation(pt_t[:], sp[:], AF.Exp, scale=LN2)
                        pt = pt_t
                    # spread the previous qtile's epilogue across the stream
                    # (earliest at p==5: the lag-AV_LAG deque finishes emitting
                    # the previous qtile's A@V matmuls during p==AV_LAG-1)
                    if pending is not None:
                        pqt, pav = pending
                        if p == 5:
                            ep_rq = ep_recip(pqt, pav)
                        elif 6 <= p <= 9:
                            ep_mul(pqt, pav, ep_rq, p - 6)
                        elif p in (10, 12):
                            ep_transpose(pqt, (p - 10) // 2)
                            if p == 12:
                                pending = None
                    if qt == NQT - 1 and p in (16, 18, 20, 22):
                        final_block((p - 16) // 2)
                    av_work.append((av, pt, p))
                    if len(av_work) > AV_LAG:
                        emit_av(av_work.popleft())
                pending = (qt, av)
            while av_work:
                emit_av(av_work.popleft())
            rq = ep_recip(*pending)
            for qc in range(4):
                ep_mul(pending[0], pending[1], rq, qc)
            for pair in range(2):
                ep_transpose(pending[0], pair)
            for pb in range(4, 8):
                final_block(pb)

    nc.compile()
    return nc


def _get_nc():
    if "nc" not in _BUILT:
        _BUILT["nc"] = _build()
    return _BUILT["nc"]


def shard_inputs(x, wq1, wq2, wk1, wk2, wv1, wv2, wo, bo):
    """Full inputs -> list of 8 per-core input maps."""
    c = np.ascontiguousarray
    x = np.asarray(x, np.float32)
    # fold attention scale and base-2 conversion into the q path
    wq1s = np.asarray(wq1, np.float32) * np.float32(D**-0.5 * LOG2E)
    import ml_dtypes

    in_maps = []
    for core in range(NCORES):
        b, g = divmod(core, 2)
        # wog bf16 [128, 4, 256]: rows 0:64 and 64:128 both hold the per-head
        # [64, 256] slice of wo for this head group
        wo_g = np.asarray(wo, np.float32)[256 * g : 256 * (g + 1), :]
        wo_h = wo_g.reshape(GH, D, QS).transpose(1, 0, 2)  # [64, 4, 256]
        wog_dup = np.concatenate([wo_h, wo_h], axis=0).astype(ml_dtypes.bfloat16)
        in_maps.append(
            {
                "xT": c(x[b].T.astype(np.float32)),
                "wq1": c(wq1s),
                "wk1": c(np.asarray(wk1, np.float32)),
                "wv1": c(np.asarray(wv1, np.float32)),
                "wq2g": c(np.asarray(wq2, np.float32)[:, 256 * g : 256 * (g + 1)]),
                "wk2": c(np.asarray(wk2, np.float32)),
                "wv2": c(np.asarray(wv2, np.float32)),
                "wog": c(wog_dup),
            }
        )
    return in_maps


def unshard_output(results, bo):
    """8 per-core partial (1024, 256) -> full (4, 1024, 256) output."""
    bo = np.asarray(bo, np.float32)
    out = np.empty((B, N, QS), np.float32)
    for b in range(B):
        out[b] = results[2 * b]["out"] + results[2 * b + 1]["out"] + bo
    return out


def kernel(x, wq1, wq2, wk1, wk2, wv1, wv2, wo, bo):
    from concourse.bass_utils import run_bass_kernel_spmd

    nc = _get_nc()
    in_maps = shard_inputs(x, wq1, wq2, wk1, wk2, wv1, wv2, wo, bo)
    res = run_bass_kernel_spmd(nc, in_maps, core_ids=list(range(NCORES)))
    return unshard_output(res.results, bo)
